# revision 11
# baseline (speedup 1.0000x reference)
"""Trainium2 Bass kernel for ClusterAssignment (vq_codebook, ALPHA=1), V3.

q[n,k] = num[n,k] / sum_k num[n,k],   num = 1/(1 + |z_n - c_k|^2)

V3 = V2 (fp8 z in, bf16 q out, 2048-row supersteps) + latency work:
  - const DMAs spread across queues (Act/Pool/DVE) so they land in parallel
  - PE p-state warmup matmuls on a zeros tile so real matmuls run fast
  - super-0 z load split in two so the first matmuls start earlier
  - per-psum-bank q stores to shorten the drain tail
  - optional bf16 num (custom-DVE reciprocal with bf16 writeback) with a
    2x halving pass before the reduce
"""

import os
import sys

if "/opt/trn_rl_repo" not in sys.path:
    sys.path.insert(0, "/opt/trn_rl_repo")

import ml_dtypes
import numpy as np

import concourse.bacc as bacc
import concourse.tile as tile
from concourse import mybir
from concourse.bass_interp import get_hw_module
from concourse.bass_utils import run_bass_kernel_spmd

N, K, D = 65536, 64, 128
NCORES = 8
NS = N // NCORES  # 8192 rows per core
NSUPER = 4
SUP_N = NS // NSUPER  # 2048
TPS = SUP_N // 128  # 16
TPB = 8
NB = 2 * TPB + 2
NBANKS = NS // (TPB * 128)  # 8

MUL_MODE = os.environ.get("V3_MUL_MODE", "pool_tt")  # pool_tt | gatings
NUM_DT = os.environ.get("V3_NUM_DT", "bf16")  # bf16 | f32
WARMUP_MM = int(os.environ.get("V3_WARMUP_MM", "10"))
UNROLL = int(os.environ.get("V3_UNROLL", "8"))  # kernel passes per For_i body

FP8 = mybir.dt.float8e4
FP8_NP = mybir.dt.np(FP8)

_CACHE = {}


def _build_nc(iters=1):
    f32 = mybir.dt.float32
    bf16 = mybir.dt.bfloat16
    num_dt = bf16 if NUM_DT == "bf16" else f32
    nc = bacc.Bacc(
        "TRN2",
        target_bir_lowering=False,
        debug=False,
        enable_asserts=False,
        num_devices=NCORES,
    )
    zT = nc.dram_tensor("zT", [D, NS], FP8, kind="ExternalInput").ap()
    cTm2 = nc.dram_tensor("cTm2", [D, K], bf16, kind="ExternalInput").ap()
    blhs = nc.dram_tensor("blhs", [NB, NBANKS * 128], bf16, kind="ExternalInput").ap()
    brhs = nc.dram_tensor("brhs", [NB, TPB * K], bf16, kind="ExternalInput").ap()
    q = nc.dram_tensor("q", [NS, K], bf16, kind="ExternalOutput").ap()

    # row n = S*2048 + p*16 + u (u in 0..15)
    q_sup = q.rearrange("(s p u) k -> s p (u k)", p=128, u=TPS)

    with tile.TileContext(nc) as tc:
        with (
            tc.tile_pool(name="const", bufs=1) as const_pool,
            tc.tile_pool(name="zin", bufs=8) as zin_pool,
            tc.tile_pool(name="num", bufs=6) as num_pool,
            tc.tile_pool(name="half", bufs=4) as half_pool,
            tc.tile_pool(name="qout", bufs=6) as q_pool,
            tc.tile_pool(name="small", bufs=6) as small_pool,
            tc.tile_pool(name="psum", bufs=4, space="PSUM") as psum_pool,
        ):
            # zeros for PE warmup: memset early on DVE (cheap, DVE idle at t=0)
            warm = const_pool.tile([128, 128], bf16)
            if WARMUP_MM:
                nc.vector.memset(warm[:], 0.0)

            # consts in parallel on three queues
            c_sb = const_pool.tile([D, K], bf16)
            nc.scalar.dma_start(c_sb[:], cTm2[:])
            blhs_sb = const_pool.tile([NB, NBANKS * 128], bf16)
            nc.gpsimd.dma_start(blhs_sb[:], blhs[:])
            brhs_sb = const_pool.tile([NB, TPB * K], bf16)
            nc.scalar.dma_start(brhs_sb[:], brhs[:])

            def z_load(s):
                zt = zin_pool.tile([D, SUP_N], FP8, tag="zt")
                if s == 0:
                    half = SUP_N // 2
                    nc.sync.dma_start(zt[:, :half], zT[:, :half])
                    nc.sync.dma_start(zt[:, half:SUP_N], zT[:, half:SUP_N])
                else:
                    nc.sync.dma_start(zt[:], zT[:, s * SUP_N : (s + 1) * SUP_N])
                return zt

            def super_compute(s, zt, first=False, coarse=False):
                ps = psum_pool.tile([128, TPS * K], f32, tag="ps")
                if first and WARMUP_MM:
                    # p-state warmup: dummy matmuls on zeros, no input deps
                    for w in range(WARMUP_MM):
                        nc.tensor.matmul(
                            ps[:, :128],
                            warm[:],
                            warm[:],
                            start=True,
                            stop=True,
                            skip_group_check=True,
                        )
                for h in range(2):
                    b = 2 * s + h
                    for t in range(TPB):
                        u = h * TPB + t
                        nc.tensor.matmul(
                            ps[:, u * K : (u + 1) * K],
                            zt[:, u * 128 : (u + 1) * 128],
                            c_sb[:],
                            start=(t == 0),
                            stop=False,
                        )
                    nc.tensor.matmul(
                        ps[:, h * TPB * K : (h + 1) * TPB * K],
                        blhs_sb[:, b * 128 : (b + 1) * 128],
                        brhs_sb[:],
                        start=False,
                        stop=True,
                    )

                num = num_pool.tile([128, TPS * K], num_dt, tag="num")
                if NUM_DT == "f32":
                    nc.vector.reciprocal_approx_fast(out=num[:], in_=ps[:])
                else:
                    from concourse.dve_ops import (
                        RECIP_APPROX_FAST_CONSTS as RC,
                        RECIPROCAL_APPROX_FAST,
                    )

                    nc.vector._custom_dve(
                        RECIPROCAL_APPROX_FAST,
                        out=num[:],
                        in0=ps[:],
                        s0=RC["s0"],
                        s1=RC["s1"],
                        imm2=RC["imm2"],
                    )

                srow = small_pool.tile([128, TPS], f32, tag="s")
                if NUM_DT == "bf16":
                    # 2x_1p halving pass, then reduce the 32-wide halves
                    nh = half_pool.tile([128, TPS * K // 2], num_dt, tag="nh")
                    nv = num[:].rearrange("p (u k) -> p u k", k=K)
                    nc.vector.tensor_add(
                        nh[:].rearrange("p (u k) -> p u k", k=K // 2),
                        nv[:, :, : K // 2],
                        nv[:, :, K // 2 :],
                    )
                    nc.vector.reduce_sum(
                        out=srow[:],
                        in_=nh[:].rearrange("p (u k) -> p u k", k=K // 2),
                        axis=mybir.AxisListType.X,
                    )
                else:
                    nc.vector.reduce_sum(
                        out=srow[:],
                        in_=num[:].rearrange("p (u k) -> p u k", k=K),
                        axis=mybir.AxisListType.X,
                    )
                sinv = small_pool.tile([128, TPS], f32, tag="sinv")
                nc.vector.reciprocal(out=sinv[:], in_=srow[:])

                qt = q_pool.tile([128, TPS * K], bf16, tag="qt")
                if coarse:
                    # unrolled steady state: one mul + one store per super
                    # (fewer gpsimd launches and DMA issues; drain amortizes)
                    nc.gpsimd.tensor_mul(
                        qt[:].rearrange("p (u k) -> p u k", k=K),
                        num[:].rearrange("p (u k) -> p u k", k=K),
                        sinv[:].broadcast_to([128, TPS, K]),
                    )
                    nc.scalar.dma_start(q_sup[s], qt[:])
                    return
                half_cols = TPB * K
                for h in range(2):
                    sl = slice(h * half_cols, (h + 1) * half_cols)
                    nc.gpsimd.tensor_mul(
                        qt[:, sl].rearrange("p (u k) -> p u k", k=K),
                        num[:, sl].rearrange("p (u k) -> p u k", k=K),
                        sinv[:, h * TPB : (h + 1) * TPB].broadcast_to(
                            [128, TPB, K]
                        ),
                    )
                    nc.scalar.dma_start(q_sup[s][:, sl], qt[:, sl])

            if MUL_MODE == "gatings":
                gat_sb = const_pool.tile([16, K // 16], f32)
                nc.vector.memset(gat_sb[:], 1.0)

            def body(first=False, coarse=False):
                zts = {}
                for s in range(NSUPER):
                    zts[s] = z_load(s)
                    super_compute(
                        s, zts[s], first=(first and s == 0), coarse=coarse
                    )

            if iters == 1:
                body(first=True)
            else:
                if WARMUP_MM:
                    ps_warm = psum_pool.tile([128, TPS * K], f32, tag="ps")
                    for w in range(WARMUP_MM):
                        nc.tensor.matmul(
                            ps_warm[:, :128],
                            warm[:],
                            warm[:],
                            start=True,
                            stop=True,
                            skip_group_check=True,
                        )
                # Unroll several full passes per loop body: the For_i
                # staggered-reset barrier drains the pipeline at every back
                # edge, so one pass per body is chain-latency-bound (~18us)
                # while no engine exceeds ~9us busy. U passes per body let
                # passes pipeline through the normal buffer rotation and
                # amortize the drain.
                u = UNROLL if iters % UNROLL == 0 and iters > UNROLL else 1
                with tc.For_i(0, iters // u, 1, staggered_reset=True):
                    for _ in range(u):
                        body(first=False, coarse=(u > 1))

    nc.compile()
    nc.m = get_hw_module(nc.m)
    return nc


def _get_nc():
    if "nc" not in _CACHE:
        _CACHE["nc"] = _build_nc()
    return _CACHE["nc"]


def _hilo(x):
    hi = x.astype(ml_dtypes.bfloat16)
    lo = (x - hi.astype(np.float64)).astype(ml_dtypes.bfloat16)
    return hi, lo


def _host_prep(z, centroids):
    z = np.asarray(z, dtype=np.float32)
    c = np.asarray(centroids, dtype=np.float32)

    cm2_bf = ((-2.0 * c.T).astype(ml_dtypes.bfloat16)).astype(np.float64)  # [D,K]
    c_eff = -0.5 * cm2_bf
    csq1 = 1.0 + (c_eff**2).sum(axis=0)
    csq1_hi, csq1_lo = _hilo(csq1)

    brhs = np.zeros((NB, TPB * K), dtype=ml_dtypes.bfloat16)
    for t in range(TPB):
        brhs[t, t * K : (t + 1) * K] = 1.0
        brhs[TPB + t, t * K : (t + 1) * K] = 1.0
    brhs[2 * TPB, :] = np.tile(csq1_hi, TPB)
    brhs[2 * TPB + 1, :] = np.tile(csq1_lo, TPB)

    in_maps = []
    for i in range(NCORES):
        zs = z[i * NS : (i + 1) * NS]
        z_perm = (
            zs.reshape(NSUPER, 128, TPS, D).transpose(0, 2, 1, 3).reshape(NS, D)
        )
        zT8 = np.ascontiguousarray(z_perm.T).astype(FP8_NP)

        z_eff = zT8.astype(np.float64).T
        zsq_perm = (z_eff**2).sum(axis=1)
        zsq_hi, zsq_lo = _hilo(zsq_perm)
        blhs = np.empty((NB, NBANKS * 128), dtype=ml_dtypes.bfloat16)
        hi = zsq_hi.reshape(NSUPER, TPS, 128).reshape(NSUPER, 2, TPB, 128)
        lo = zsq_lo.reshape(NSUPER, TPS, 128).reshape(NSUPER, 2, TPB, 128)
        blhs[:TPB] = hi.transpose(2, 0, 1, 3).reshape(TPB, -1)
        blhs[TPB : 2 * TPB] = lo.transpose(2, 0, 1, 3).reshape(TPB, -1)
        blhs[2 * TPB :] = 1.0
        in_maps.append(
            {
                "zT": zT8,
                "cTm2": cm2_bf.astype(ml_dtypes.bfloat16),
                "blhs": blhs,
                "brhs": brhs,
            }
        )
    return in_maps


def kernel(z, centroids):
    nc = _get_nc()
    in_maps = _host_prep(z, centroids)
    res = run_bass_kernel_spmd(nc, in_maps, list(range(NCORES)))
    out = np.concatenate(
        [np.asarray(res.results[i]["q"]) for i in range(NCORES)], axis=0
    )
    return out.astype(np.float32)
